# revision 2
# baseline (speedup 1.0000x reference)
"""GNN message passing on 8 trn2 NeuronCores.

out = relu(segment_sum_tgt(X[src] @ W_l))  with  X:[50000,512] f32,
adjacency:[4,40000,2] i32, W:[4,512,512] f32.

Strategy: shard by TARGET node (core c owns output rows [c*6250,(c+1)*6250))
so no cross-core reduction is needed.  Per core, edges are grouped on the
host by (node-tile k of 128 rows, edge type l) into 128-slot chunks.

Per (k, l):   Yt(l)[d, v] = sum_e Xg[e, d] * Ind[e, v]     (PE, bf16)
  where Xg = gathered source rows (indirect DMA from bf16 X),
  Ind[e, v] = (tgt_local[e] == v)                           (VectorE)
Per tile k:   out[v, h] = relu( sum_{l,dt} Yt(l)[dt]^T @ W[l,dt] )  (PE)

All cores run the same program (SPMD); chunk counts are the max over
cores, with pad slots (src=0, tgt=-1) contributing exactly zero.
"""

import os
import sys

sys.path.insert(0, "/opt/trn_rl_repo")

import ml_dtypes
import numpy as np

V, D, H, L, E = 50000, 512, 512, 4, 40000
NCORES = 8
VC = V // NCORES  # 6250 output rows per core
P = 128
NT = (VC + P - 1) // P  # 49 node tiles per core
LAST_ROWS = VC - (NT - 1) * P  # 106

LAST_RESULTS = None  # BassKernelResults of the most recent run (for test.py)


def _build_schedule(adjacency):
    """Group edges by (core, node-tile, type); return the shared static
    chunk schedule plus per-core slot arrays."""
    src = np.asarray(adjacency[..., 0], dtype=np.int64)  # [L, E]
    tgt = np.asarray(adjacency[..., 1], dtype=np.int64)  # [L, E]
    core = tgt // VC
    tl = tgt - core * VC  # local row in core slice
    kk = tl // P  # node tile index
    vloc = (tl - kk * P).astype(np.float32)  # 0..127 within tile

    counts = np.zeros((NCORES, NT, L), dtype=np.int64)
    for l in range(L):
        np.add.at(counts, (core[l], kk[l], l), 1)
    maxcnt = counts.max(axis=0)  # [NT, L]
    chunks = np.maximum(1, -(-maxcnt // P)).astype(np.int64)  # [NT, L]

    ck_tile = chunks.sum(axis=1)  # [NT]
    tile_base = np.zeros(NT, dtype=np.int64)
    tile_base[1:] = np.cumsum(ck_tile)[:-1]
    col_base = np.zeros((NT, L), dtype=np.int64)  # first column of (k,l)
    for k in range(NT):
        acc = tile_base[k]
        for l in range(L):
            col_base[k, l] = acc
            acc += chunks[k, l]
    C_total = int(ck_tile.sum())

    srcs_T = np.zeros((NCORES, P, C_total), dtype=np.int32)
    tgtv_T = np.full((NCORES, P, C_total), -1.0, dtype=np.float32)
    for c in range(NCORES):
        for l in range(L):
            sel = core[l] == c
            kk_c = kk[l][sel]
            src_c = src[l][sel]
            v_c = vloc[l][sel]
            order = np.argsort(kk_c, kind="stable")
            kk_s = kk_c[order]
            src_s = src_c[order]
            v_s = v_c[order]
            grp_start = np.zeros(NT, dtype=np.int64)
            grp_start[1:] = np.cumsum(np.bincount(kk_s, minlength=NT))[:-1]
            pos = np.arange(len(kk_s)) - grp_start[kk_s]
            col = col_base[kk_s, l] + pos // P
            row = pos % P
            srcs_T[c, row, col] = src_s.astype(np.int32)
            tgtv_T[c, row, col] = v_s
    return chunks, col_base, tile_base, ck_tile, C_total, srcs_T, tgtv_T


def _build_program(chunks, col_base, tile_base, ck_tile, C_total):
    import concourse.bacc as bacc
    import concourse.mybir as mybir
    import concourse.tile as tile
    from concourse.bass import IndirectOffsetOnAxis

    nc = bacc.Bacc(
        "TRN2", target_bir_lowering=False, debug=False, num_devices=NCORES
    )
    bf16 = mybir.dt.bfloat16
    f32 = mybir.dt.float32
    i32 = mybir.dt.int32

    xbf = nc.dram_tensor("xbf", [V, D], bf16, kind="ExternalInput").ap()
    wsb_in = nc.dram_tensor("wsb", [P, L * 4 * H], bf16, kind="ExternalInput").ap()
    iota_in = nc.dram_tensor("iota", [P, P], f32, kind="ExternalInput").ap()
    srcs = nc.dram_tensor("srcs", [P, C_total], i32, kind="ExternalInput").ap()
    tgtv = nc.dram_tensor("tgtv", [P, C_total], f32, kind="ExternalInput").ap()
    outt = nc.dram_tensor("out", [VC, H], f32, kind="ExternalOutput").ap()

    ck_max = int(ck_tile.max())

    with tile.TileContext(nc) as tc:
        with (
            tc.tile_pool(name="const", bufs=1) as constp,
            tc.tile_pool(name="idx", bufs=3) as idxp,
            tc.tile_pool(name="xg", bufs=10) as xgp,
            tc.tile_pool(name="ind", bufs=10) as indp,
            tc.tile_pool(name="yts", bufs=3) as ytsp,
            tc.tile_pool(name="outs", bufs=3) as outsp,
            tc.tile_pool(name="yt", bufs=2, space="PSUM") as ytp,
            tc.tile_pool(name="accp", bufs=2, space="PSUM") as accp,
        ):
            w_s = constp.tile([P, L * 4 * H], bf16)
            nc.sync.dma_start(out=w_s[:], in_=wsb_in[:])
            iota_s = constp.tile([P, P], f32)
            nc.sync.dma_start(out=iota_s[:], in_=iota_in[:])

            for k in range(NT):
                ck = int(ck_tile[k])
                base = int(tile_base[k])
                src_sb = idxp.tile([P, ck_max], i32, tag="src_sb")
                nc.sync.dma_start(
                    out=src_sb[:, :ck], in_=srcs[:, base : base + ck]
                )
                tgt_sb = idxp.tile([P, ck_max], f32, tag="tgt_sb")
                nc.sync.dma_start(
                    out=tgt_sb[:, :ck], in_=tgtv[:, base : base + ck]
                )

                acc = accp.tile([P, H], f32)
                mm_i = 0
                for l in range(L):
                    nch = int(chunks[k, l])
                    c0 = int(col_base[k, l]) - base  # local column offset
                    xgs = []
                    inds = []
                    for c in range(nch):
                        col = c0 + c
                        xg = xgp.tile([P, D], bf16, tag="xg")
                        nc.gpsimd.indirect_dma_start(
                            out=xg[:],
                            out_offset=None,
                            in_=xbf[:],
                            in_offset=IndirectOffsetOnAxis(
                                ap=src_sb[:, col : col + 1], axis=0
                            ),
                        )
                        ind = indp.tile([P, P], bf16, tag="ind")
                        nc.vector.tensor_tensor(
                            out=ind[:],
                            in0=tgt_sb[:, col : col + 1].to_broadcast([P, P]),
                            in1=iota_s[:],
                            op=mybir.AluOpType.is_equal,
                        )
                        xgs.append(xg)
                        inds.append(ind)

                    yt = ytp.tile([P, 4 * P], f32)  # [d-in-tile, 4 x v] one bank
                    n_mm = 4 * nch
                    i_mm = 0
                    for c in range(nch):
                        for dt in range(4):
                            nc.tensor.matmul(
                                out=yt[:, dt * P : (dt + 1) * P],
                                lhsT=xgs[c][:, dt * P : (dt + 1) * P],
                                rhs=inds[c][:],
                                start=(i_mm == 0),
                                stop=(i_mm == n_mm - 1),
                            )
                            i_mm += 1

                    yts = ytsp.tile([P, 4 * P], bf16, tag="yts")
                    nc.vector.tensor_copy(out=yts[:], in_=yt[:])

                    for dt in range(4):
                        q = l * 4 + dt
                        nc.tensor.matmul(
                            out=acc[:],
                            lhsT=yts[:, dt * P : (dt + 1) * P],
                            rhs=w_s[:, q * H : (q + 1) * H],
                            start=(mm_i == 0),
                            stop=(mm_i == 4 * L - 1),
                        )
                        mm_i += 1

                rows = P if k < NT - 1 else LAST_ROWS
                o = outsp.tile([P, H], f32, tag="o")
                nc.scalar.activation(
                    out=o[:rows],
                    in_=acc[:rows],
                    func=mybir.ActivationFunctionType.Relu,
                )
                nc.sync.dma_start(
                    out=outt[k * P : k * P + rows, :], in_=o[:rows]
                )

    nc.compile()
    return nc


def kernel(node_embeddings, adjacency, W):
    global LAST_RESULTS
    from concourse.bass_utils import run_bass_kernel_spmd

    x = np.ascontiguousarray(np.asarray(node_embeddings, dtype=np.float32))
    adj = np.asarray(adjacency, dtype=np.int32)
    w = np.asarray(W, dtype=np.float32)

    xbf = x.astype(ml_dtypes.bfloat16)
    # Wsb[p, (l*4+dt)*H + h] = W[l, dt*128+p, h]
    wsb = np.ascontiguousarray(
        w.reshape(L, 4, P, H).transpose(2, 0, 1, 3).reshape(P, L * 4 * H)
    ).astype(ml_dtypes.bfloat16)
    iota = np.tile(np.arange(P, dtype=np.float32), (P, 1))
    iota = np.ascontiguousarray(iota)

    chunks, col_base, tile_base, ck_tile, C_total, srcs_T, tgtv_T = (
        _build_schedule(adj)
    )
    nc = _build_program(chunks, col_base, tile_base, ck_tile, C_total)

    in_maps = [
        {
            "xbf": xbf,
            "wsb": wsb,
            "iota": iota,
            "srcs": np.ascontiguousarray(srcs_T[c]),
            "tgtv": np.ascontiguousarray(tgtv_T[c]),
        }
        for c in range(NCORES)
    ]
    tmpdir = os.environ.get("KERNEL_TMPDIR")
    if tmpdir:
        import shutil
        import uuid

        tmpdir = os.path.join(tmpdir, uuid.uuid4().hex[:8])
        shutil.rmtree(tmpdir, ignore_errors=True)
        os.makedirs(tmpdir, exist_ok=True)
    res = run_bass_kernel_spmd(
        nc,
        in_maps,
        list(range(NCORES)),
        tmpdir=tmpdir,
    )
    LAST_RESULTS = res
    out = np.concatenate(
        [np.asarray(res.results[c]["out"]) for c in range(NCORES)], axis=0
    )
    return out.astype(np.float32)


# revision 6
# speedup vs baseline: 1.0240x; 1.0240x over previous
"""GNN message passing on 8 trn2 NeuronCores.

out = relu(segment_sum_tgt(X[src] @ W_l))  with  X:[50000,512] f32,
adjacency:[4,40000,2] i32, W:[4,512,512] f32.

Strategy: shard by TARGET node (core c owns output rows [c*6250,(c+1)*6250))
so no cross-core reduction is needed.  Per core, edges are grouped on the
host by (node-tile k of 128 rows, edge type l) into 128-slot chunks.

Per (k, l):   Yt(l)[d, v] = sum_e Xg[e, d] * Ind[e, v]     (PE, bf16)
  where Xg = gathered source rows (indirect DMA from bf16 X),
  Ind[e, v] = (tgt_local[e] == v)                           (VectorE)
Per tile k:   out[v, h] = relu( sum_{l,dt} Yt(l)[dt]^T @ W[l,dt] )  (PE)

All cores run the same program (SPMD); chunk counts are the max over
cores, with pad slots (src=0, tgt=-1) contributing exactly zero.
"""

import os
import sys

sys.path.insert(0, "/opt/trn_rl_repo")

import ml_dtypes
import numpy as np

V, D, H, L, E = 50000, 512, 512, 4, 40000
NCORES = 8
VC = V // NCORES  # 6250 output rows per core
P = 128
NT = (VC + P - 1) // P  # 49 node tiles per core
LAST_ROWS = VC - (NT - 1) * P  # 106

LAST_RESULTS = None  # BassKernelResults of the most recent run (for test.py)


def _build_schedule(adjacency):
    """Group edges by (core, node-tile, type); return the shared static
    chunk schedule plus per-core slot arrays."""
    src = np.asarray(adjacency[..., 0], dtype=np.int64)  # [L, E]
    tgt = np.asarray(adjacency[..., 1], dtype=np.int64)  # [L, E]
    core = tgt // VC
    tl = tgt - core * VC  # local row in core slice
    kk = tl // P  # node tile index
    vloc = (tl - kk * P).astype(np.float32)  # 0..127 within tile

    counts = np.zeros((NCORES, NT, L), dtype=np.int64)
    for l in range(L):
        np.add.at(counts, (core[l], kk[l], l), 1)
    maxcnt = counts.max(axis=0)  # [NT, L]
    chunks = np.maximum(1, -(-maxcnt // P)).astype(np.int64)  # [NT, L]

    ck_tile = chunks.sum(axis=1)  # [NT]
    tile_base = np.zeros(NT, dtype=np.int64)
    tile_base[1:] = np.cumsum(ck_tile)[:-1]
    col_base = np.zeros((NT, L), dtype=np.int64)  # first column of (k,l)
    for k in range(NT):
        acc = tile_base[k]
        for l in range(L):
            col_base[k, l] = acc
            acc += chunks[k, l]
    C_total = int(ck_tile.sum())

    srcs_T = np.zeros((NCORES, P, C_total), dtype=np.int32)
    tgtv_T = np.full((NCORES, P, C_total), -1.0, dtype=np.float32)
    for c in range(NCORES):
        for l in range(L):
            sel = core[l] == c
            kk_c = kk[l][sel]
            src_c = src[l][sel]
            v_c = vloc[l][sel]
            order = np.argsort(kk_c, kind="stable")
            kk_s = kk_c[order]
            src_s = src_c[order]
            v_s = v_c[order]
            grp_start = np.zeros(NT, dtype=np.int64)
            grp_start[1:] = np.cumsum(np.bincount(kk_s, minlength=NT))[:-1]
            pos = np.arange(len(kk_s)) - grp_start[kk_s]
            col = col_base[kk_s, l] + pos // P
            row = pos % P
            srcs_T[c, row, col] = src_s.astype(np.int32)
            tgtv_T[c, row, col] = v_s
    return chunks, col_base, tile_base, ck_tile, C_total, srcs_T, tgtv_T


def _build_program(chunks, col_base, tile_base, ck_tile, C_total):
    import concourse.bacc as bacc
    import concourse.mybir as mybir
    import concourse.tile as tile
    from concourse.bass import IndirectOffsetOnAxis

    nc = bacc.Bacc(
        "TRN2", target_bir_lowering=False, debug=False, num_devices=NCORES
    )
    bf16 = mybir.dt.bfloat16
    f32 = mybir.dt.float32
    i32 = mybir.dt.int32

    xbf = nc.dram_tensor("xbf", [V, D], bf16, kind="ExternalInput").ap()
    wsb_in = nc.dram_tensor("wsb", [P, L * 4 * H], bf16, kind="ExternalInput").ap()
    iota_in = nc.dram_tensor("iota", [P, P], f32, kind="ExternalInput").ap()
    srcs = nc.dram_tensor("srcs", [P, C_total], i32, kind="ExternalInput").ap()
    tgtv = nc.dram_tensor("tgtv", [P, C_total], f32, kind="ExternalInput").ap()
    outt = nc.dram_tensor("out", [VC, H], f32, kind="ExternalOutput").ap()

    ck_max = int(ck_tile.max())

    with tile.TileContext(nc) as tc:
        with (
            tc.tile_pool(name="const", bufs=1) as constp,
            tc.tile_pool(name="idx", bufs=4) as idxp,
            tc.tile_pool(name="xg", bufs=12) as xgp,
            tc.tile_pool(name="ind", bufs=12) as indp,
            tc.tile_pool(name="yts", bufs=4) as ytsp,
            tc.tile_pool(name="outs", bufs=3) as outsp,
            tc.tile_pool(name="yt", bufs=3, space="PSUM") as ytp,
            tc.tile_pool(name="accp", bufs=3, space="PSUM") as accp,
        ):
            w_s = constp.tile([P, L * 4 * H], bf16)
            nc.sync.dma_start(out=w_s[:], in_=wsb_in[:])
            iota_s = constp.tile([P, P], f32)
            nc.sync.dma_start(out=iota_s[:], in_=iota_in[:])

            for k in range(NT):
                ck = int(ck_tile[k])
                base = int(tile_base[k])
                src_sb = idxp.tile([P, ck_max], i32, tag="src_sb")
                nc.sync.dma_start(
                    out=src_sb[:, :ck], in_=srcs[:, base : base + ck]
                )
                tgt_sb = idxp.tile([P, ck_max], f32, tag="tgt_sb")
                nc.sync.dma_start(
                    out=tgt_sb[:, :ck], in_=tgtv[:, base : base + ck]
                )

                acc = accp.tile([P, H], f32)
                mm_i = 0
                for l in range(L):
                    nch = int(chunks[k, l])
                    c0 = int(col_base[k, l]) - base  # local column offset
                    xgs = []
                    inds = []
                    for c in range(nch):
                        col = c0 + c
                        xg = xgp.tile([P, D], bf16, tag="xg")
                        nc.gpsimd.indirect_dma_start(
                            out=xg[:],
                            out_offset=None,
                            in_=xbf[:],
                            in_offset=IndirectOffsetOnAxis(
                                ap=src_sb[:, col : col + 1], axis=0
                            ),
                        )
                        ind = indp.tile([P, P], bf16, tag="ind")
                        nc.vector.tensor_tensor(
                            out=ind[:],
                            in0=tgt_sb[:, col : col + 1].to_broadcast([P, P]),
                            in1=iota_s[:],
                            op=mybir.AluOpType.is_equal,
                        )
                        xgs.append(xg)
                        inds.append(ind)

                    yt = ytp.tile([P, 4 * P], f32)  # [d-in-tile, 4 x v] one bank
                    n_mm = 4 * nch
                    i_mm = 0
                    for c in range(nch):
                        for dt in range(4):
                            nc.tensor.matmul(
                                out=yt[:, dt * P : (dt + 1) * P],
                                lhsT=xgs[c][:, dt * P : (dt + 1) * P],
                                rhs=inds[c][:],
                                start=(i_mm == 0),
                                stop=(i_mm == n_mm - 1),
                            )
                            i_mm += 1

                    yts = ytsp.tile([P, 4 * P], bf16, tag="yts")
                    # cast on the (otherwise idle) Scalar engine, not Vector
                    nc.scalar.activation(
                        out=yts[:],
                        in_=yt[:],
                        func=mybir.ActivationFunctionType.Copy,
                    )

                    for dt in range(4):
                        q = l * 4 + dt
                        nc.tensor.matmul(
                            out=acc[:],
                            lhsT=yts[:, dt * P : (dt + 1) * P],
                            rhs=w_s[:, q * H : (q + 1) * H],
                            start=(mm_i == 0),
                            stop=(mm_i == 4 * L - 1),
                        )
                        mm_i += 1

                rows = P if k < NT - 1 else LAST_ROWS
                o = outsp.tile([P, H], f32, tag="o")
                nc.scalar.activation(
                    out=o[:rows],
                    in_=acc[:rows],
                    func=mybir.ActivationFunctionType.Relu,
                )
                nc.sync.dma_start(
                    out=outt[k * P : k * P + rows, :], in_=o[:rows]
                )

    nc.compile()
    return nc


def kernel(node_embeddings, adjacency, W):
    global LAST_RESULTS
    from concourse.bass_utils import run_bass_kernel_spmd

    x = np.ascontiguousarray(np.asarray(node_embeddings, dtype=np.float32))
    adj = np.asarray(adjacency, dtype=np.int32)
    w = np.asarray(W, dtype=np.float32)

    xbf = x.astype(ml_dtypes.bfloat16)
    # Wsb[p, (l*4+dt)*H + h] = W[l, dt*128+p, h]
    wsb = np.ascontiguousarray(
        w.reshape(L, 4, P, H).transpose(2, 0, 1, 3).reshape(P, L * 4 * H)
    ).astype(ml_dtypes.bfloat16)
    iota = np.tile(np.arange(P, dtype=np.float32), (P, 1))
    iota = np.ascontiguousarray(iota)

    chunks, col_base, tile_base, ck_tile, C_total, srcs_T, tgtv_T = (
        _build_schedule(adj)
    )
    nc = _build_program(chunks, col_base, tile_base, ck_tile, C_total)

    in_maps = [
        {
            "xbf": xbf,
            "wsb": wsb,
            "iota": iota,
            "srcs": np.ascontiguousarray(srcs_T[c]),
            "tgtv": np.ascontiguousarray(tgtv_T[c]),
        }
        for c in range(NCORES)
    ]
    tmpdir = os.environ.get("KERNEL_TMPDIR")
    if tmpdir:
        import shutil
        import uuid

        tmpdir = os.path.join(tmpdir, uuid.uuid4().hex[:8])
        shutil.rmtree(tmpdir, ignore_errors=True)
        os.makedirs(tmpdir, exist_ok=True)
    res = run_bass_kernel_spmd(
        nc,
        in_maps,
        list(range(NCORES)),
        tmpdir=tmpdir,
    )
    LAST_RESULTS = res
    out = np.concatenate(
        [np.asarray(res.results[c]["out"]) for c in range(NCORES)], axis=0
    )
    return out.astype(np.float32)
